# revision 36
# baseline (speedup 1.0000x reference)
"""DenseCapsule dynamic-routing kernel for 8 Trainium2 NeuronCores.

Problem: x[B=32,I=2048,D=16], w_ij[J=64,I=2048,C=32,D=16]
  u_hat = einsum('bid,jicd->bjic', x, w_ij)
  5 routing iterations (softmax over J, s = sum_i c*u_hat, v = squash(s),
  b += sum_c v*u_hat), return v [B,J,C].

Sharding: input capsules I are split 8 ways (I_LOC=256 per core).  The
softmax over J is then core-local; the only collective is an AllReduce of
the per-core partial s [B,J,C] (128 KB fp16) once per iteration.

v4: u_hat is never stored — each routing iteration recomputes it from W
on the PE (one extra 512KB wm stream per group, 64MB/core total) instead
of the v0 scheme's 32MB u_store write + 128MB of re-reads.  Phase 1
shrinks to just the iteration-1 s-pass (s^1 = sum_i u/J via the stacked-x
stationary, no evacuations).  Free axes are (c, j) [v3]: j innermost
lets DVE's prod2 read the softmax weights through a stride-0 middle-axis
broadcast view of e_grp in 2x mode (no e_rep materialization), 1/Z rides
the PE stationary ln_n = d1 * invz_n built on ACT, and the logit-update
c-reduction is a pairwise fp16 tree on DVE (all levels 2x).  Host
transposes the [B,(C,J)] result back to [B,J,C].
"""

import numpy as np

B, I, D, J, C = 32, 2048, 16, 64, 32
NCORES = 8
I_LOC = I // NCORES      # 256
G = 4                    # i's per block (G*D = 64 contraction partitions)
NBLK = I_LOC // G        # 64
JC = J * C               # 2048 (free size; layout is (c, j) in-kernel)
ITERS = 5
EPS = 1e-7
NCH = 4                  # 512-wide matmul chunks over the (c,j) free axis
GR = 4                   # i-blocks per phase-2 tile group
NG = NBLK // GR          # 16 groups per iteration

_CACHE = {}


def _build(repeats=1):
    import concourse.bacc as bacc
    import concourse.mybir as mybir
    from concourse import tile

    f32 = mybir.dt.float32
    fp16 = mybir.dt.float16
    bf16 = mybir.dt.bfloat16
    Act = mybir.ActivationFunctionType
    Alu = mybir.AluOpType
    X = mybir.AxisListType.X

    nc = bacc.Bacc("TRN2", target_bir_lowering=False, debug=False,
                   num_devices=NCORES)
    NP = NBLK // 2   # block pairs: two G*D=64 blocks stacked on 128 partitions
    xd = nc.dram_tensor("xd", [NP, 128, 128], fp16, kind="ExternalInput").ap()
    xs = nc.dram_tensor("xs", [NP, 128, B], fp16, kind="ExternalInput").ap()
    wm = nc.dram_tensor("wm", [NP, 128, JC], fp16, kind="ExternalInput").ap()
    d1 = nc.dram_tensor("d1", [128, B], fp16, kind="ExternalInput").ap()
    v_out = nc.dram_tensor("v_out", [B, JC], f32, kind="ExternalOutput").ap()

    with tile.TileContext(nc) as tc:
        with tc.tile_pool(name="const", bufs=1) as constp, \
             tc.tile_pool(name="io", bufs=2) as iop, \
             tc.tile_pool(name="u", bufs=1) as up, \
             tc.tile_pool(name="work", bufs=2) as wp, \
             tc.tile_pool(name="small", bufs=1) as sp, \
             tc.tile_pool(name="spg", bufs=3) as spg, \
             tc.tile_pool(name="psum", bufs=4, space="PSUM") as pp, \
             tc.tile_pool(name="spsum", bufs=1, space="PSUM") as spp, \
             tc.tile_pool(name="ar", bufs=2, space="DRAM") as arp:

            d1_t = constp.tile([128, B], fp16)
            nc.sync.dma_start(d1_t[:], d1[:])
            eps_t = constp.tile([128, 1], f32, tag="eps")
            nc.gpsimd.memset(eps_t[:], EPS)
            # x stays resident: block-diag form (1MB) + stacked form (256KB)
            xd_all = constp.tile([128, NP * 128], fp16, tag="xd_all")
            nc.sync.dma_start(
                xd_all[:].rearrange("p (n f) -> p n f", n=NP),
                xd[:].rearrange("n p f -> p n f"))
            xs_all = constp.tile([128, NP * B], fp16, tag="xs_all")
            nc.sync.dma_start(
                xs_all[:].rearrange("p (n f) -> p n f", n=NP),
                xs[:].rearrange("n p f -> p n f"))
            b_tiles = []                                 # routing logits
            for gi in range(NG):
                bt = constp.tile([128, GR * J], f32, tag=f"b{gi}")
                b_tiles.append(bt)

            for _rep in range(repeats):
                for bt in b_tiles:
                    nc.gpsimd.memset(bt[:], 0.0)

                # ---- Phase 1: iteration-1 s accumulation only.
                # s^1 = sum_i u/J comes straight from x,W with the stacked-x
                # stationary (no block-diagonal, no u materialization): pure
                # PE streaming over wm.
                s_ps = spp.tile([B, JC], f32, tag="s")
                for pr in range(NP):
                    wm_t = iop.tile([128, JC], fp16, tag="wm_t", bufs=5)
                    nc.sync.dma_start(wm_t[:, :JC // 2], wm[pr][:, :JC // 2])
                    nc.sync.dma_start(wm_t[:, JC // 2:], wm[pr][:, JC // 2:])
                    for ch in range(NCH):
                        sl = slice(ch * 512, (ch + 1) * 512)
                        nc.tensor.matmul(s_ps[:, sl],
                                         xs_all[:, pr * B:(pr + 1) * B],
                                         wm_t[:, sl],
                                         start=(pr == 0), stop=(pr == NP - 1))

                # ---- Phase 2: routing iterations
                for it in range(1, ITERS + 1):
                    # v^{it} from the s accumulated for iteration `it`.
                    # AllReduce payload in fp16 (halves collective bytes; the
                    # 8-way partial sums tolerate fp16 rounding).
                    s_sb = sp.tile([B, JC], fp16, tag="s_sb")
                    nc.scalar.activation(s_sb[:], s_ps[:], Act.Copy, bias=0.0)
                    ar_in = arp.tile([B, JC], fp16, tag="ar_in")
                    ar_out = arp.tile([B, JC], fp16, tag="ar_out")
                    nc.sync.dma_start(ar_in[:], s_sb[:])
                    nc.gpsimd.collective_compute(
                        "AllReduce", Alu.add,
                        replica_groups=[list(range(NCORES))],
                        ins=[ar_in.opt()], outs=[ar_out.opt()],
                    )
                    # AR return replicated onto all 4 partition-quarters:
                    # the squash then computes v directly in the 128-partition
                    # layout prod1 needs (no separate v_rep step)
                    s_full = sp.tile([128, JC], fp16, tag="s_full")
                    for g in range(G):
                        nc.sync.dma_start(s_full[g * B:(g + 1) * B, :],
                                          ar_out[:])

                    # squash: v = (s+eps) * scale, scale = sqrt(n)/(1+n),
                    # n = sum_c (s+eps)^2.  sq on ACT (fused +EPS bias); the
                    # c-reduction is a contiguous fp16 halving tree (2x mode,
                    # c is the outer free axis).
                    sq = sp.tile([128, JC], fp16, tag="sq")
                    nc.vector.scalar_tensor_tensor(
                        sq[:], s_full[:], EPS, s_full[:],
                        op0=Alu.add, op1=Alu.mult)
                    h = JC // 2
                    while h >= J:
                        nc.vector.tensor_add(sq[:, 0:h], sq[:, 0:h],
                                             sq[:, h:2 * h])
                        h //= 2
                    norm = sp.tile([128, J], f32, tag="norm")
                    nc.vector.tensor_copy(norm[:], sq[:, 0:J])
                    rt = sp.tile([128, J], f32, tag="rt")
                    nc.scalar.activation(rt[:], norm[:], Act.Sqrt)
                    warm = sp.tile([128, 1], f32, tag="warm")
                    nc.scalar.activation(warm[:], eps_t[:], Act.Exp)
                    np1 = sp.tile([128, J], f32, tag="np1")
                    nc.vector.tensor_scalar_add(np1[:], norm[:], 1.0)
                    inv1 = sp.tile([128, J], f32, tag="inv1")
                    nc.vector.reciprocal_approx_fast(inv1[:], np1[:])
                    if it == ITERS:
                        invd = sp.tile([128, J], f32, tag="invd")
                        nc.vector.tensor_mul(invd[:], rt[:], inv1[:])
                        v_sb = sp.tile([B, JC], f32, tag="v0")
                        nc.vector.scalar_tensor_tensor(
                            v_sb[:].rearrange("p (c j) -> p c j", j=J),
                            s_full[0:B, :].rearrange("p (c j) -> p c j", j=J),
                            EPS,
                            invd[0:B, :].rearrange("p (one j) -> p one j",
                                                   one=1)
                                        .broadcast_to((B, C, J)),
                            op0=Alu.add, op1=Alu.mult)
                        nc.sync.dma_start(v_out[:], v_sb[:])
                        break

                    # v16 = (s+eps)*invd fused in one DVE pass; fp16 invd
                    # keeps every operand 2B so the STT runs in 2x mode
                    invd = sp.tile([128, J], fp16, tag="invd")
                    nc.vector.tensor_mul(invd[:], rt[:], inv1[:])
                    v_rep = constp.tile([128, JC], fp16, tag="v_rep")
                    nc.vector.scalar_tensor_tensor(
                        v_rep[:].rearrange("p (c j) -> p c j", j=J),
                        s_full[:].rearrange("p (c j) -> p c j", j=J),
                        EPS,
                        invd[:].rearrange("p (one j) -> p one j", one=1)
                               .broadcast_to((128, C, J)),
                        op0=Alu.add, op1=Alu.mult)

                    s_ps = spp.tile([B, JC], f32, tag="s")

                    def stage_b(u_t, e_grp, zacc, g0):
                        # s += (e/Z) * u: e applied on DVE through a broadcast
                        # view of e_grp (middle-axis stride-0, innermost j
                        # unit-stride -> still 2x); 1/Z rides the stationary.
                        # ln built here (a group later than its recip) so the
                        # in-order ACT queue never stalls waiting on DVE.
                        invz = spg.tile([128, GR], f32, tag="invz", bufs=5)
                        nc.vector.reciprocal(invz[:], zacc[:])
                        lns = []
                        for n in range(GR):
                            ln = spg.tile([128, B], bf16, tag=f"ln{n}", bufs=5,
                                          name=f"ln{n}")
                            nc.scalar.mul(ln[:], d1_t[:], invz[:, n:n + 1])
                            lns.append(ln)
                        prod2 = wp.tile([128, GR * JC], bf16, tag="prod1", bufs=3)
                        nc.vector.tensor_mul(
                            prod2[:].rearrange("p (n c j) -> p n c j",
                                               n=GR, j=J),
                            u_t[:].rearrange("p (n c j) -> p n c j",
                                             n=GR, j=J),
                            e_grp[:].rearrange("p (n o j) -> p n o j",
                                               n=GR, o=1)
                                    .broadcast_to((128, GR, C, J)))
                        for n in range(GR):
                            blk = g0 + n
                            for ch in range(NCH):
                                sl = slice(n * JC + ch * 512,
                                           n * JC + (ch + 1) * 512)
                                nc.tensor.matmul(s_ps[:, ch * 512:(ch + 1) * 512],
                                                 lns[n][:], prod2[:, sl],
                                                 start=(blk == 0),
                                                 stop=(blk == NBLK - 1))

                    def produce_u(gi):
                        # recompute group gi's u_hat from W on the PE:
                        # 2 block-pairs, 4 chunks each, evacuated by ACT into
                        # the fp16 u_t working tile (no DRAM round-trip).
                        u_t = up.tile([128, GR * JC], fp16, tag="u_t", bufs=5)
                        for h in range(2):
                            pr = 2 * gi + h
                            wm_t = iop.tile([128, JC], fp16, tag="wm_t", bufs=5)
                            nc.sync.dma_start(wm_t[:, :JC // 2],
                                              wm[pr][:, :JC // 2])
                            nc.sync.dma_start(wm_t[:, JC // 2:],
                                              wm[pr][:, JC // 2:])
                            xsl = xd_all[:, pr * 128:(pr + 1) * 128]
                            for ch in range(NCH):
                                sl = slice(ch * 512, (ch + 1) * 512)
                                psa = pp.tile([128, 512], f32, tag="psa", bufs=2)
                                nc.tensor.matmul(psa[:], xsl[0:64, :],
                                                 wm_t[0:64, sl],
                                                 start=True, stop=True,
                                                 tile_position=(0, 0))
                                psb = pp.tile([128, 512], f32, tag="psb", bufs=2)
                                nc.tensor.matmul(psb[:], xsl[64:128, :],
                                                 wm_t[64:128, sl],
                                                 start=True, stop=True,
                                                 tile_position=(64, 0))
                                nc.scalar.copy(
                                    u_t[:, (2 * h) * JC + ch * 512:
                                        (2 * h) * JC + (ch + 1) * 512], psa[:])
                                nc.scalar.copy(
                                    u_t[:, (2 * h + 1) * JC + ch * 512:
                                        (2 * h + 1) * JC + (ch + 1) * 512],
                                    psb[:])
                        return u_t

                    # u-production runs one group ahead so the PE queue does
                    # u-mms(g+1) before s-mms(g-1): the critical cycle
                    # prod2->s-mms->u-mms->evacs->prod1 loses the PE+ACT leg
                    pending = None
                    u_q = [produce_u(0), produce_u(1)]
                    for g0 in range(0, NBLK, GR):
                        gi = g0 // GR
                        b_g = b_tiles[gi]
                        u_t = u_q.pop(0)
                        if gi + 2 < NG:
                            u_q.append(produce_u(gi + 2))
                        # logits update t = sum_c u*v: DVE mul + pairwise fp16
                        # tree over the outer c axis (all levels contiguous
                        # 64-wide j-runs -> 2x mode).
                        prod1 = wp.tile([128, GR * JC], fp16, tag="prod1", bufs=3)
                        nc.vector.tensor_mul(
                            prod1[:].rearrange("p (n f) -> p n f", n=GR),
                            u_t[:].rearrange("p (n f) -> p n f", n=GR),
                            v_rep[:].rearrange("p (o f) -> p o f", o=1)
                                    .broadcast_to((128, GR, JC)))
                        p4 = prod1[:].rearrange("p (n c j) -> p n c j",
                                                n=GR, j=J)
                        nc.vector.tensor_add(p4[:, :, 0:16, :], p4[:, :, 0:16, :],
                                             p4[:, :, 16:32, :])
                        nc.vector.tensor_add(p4[:, :, 0:8, :], p4[:, :, 0:8, :],
                                             p4[:, :, 8:16, :])
                        nc.vector.tensor_add(p4[:, :, 0:4, :], p4[:, :, 0:4, :],
                                             p4[:, :, 4:8, :])
                        nc.vector.tensor_add(p4[:, :, 0:2, :], p4[:, :, 0:2, :],
                                             p4[:, :, 2:4, :])
                        t16 = spg.tile([128, GR * J], fp16, tag="t16", bufs=5)
                        nc.vector.tensor_add(
                            t16[:].rearrange("p (n o j) -> p n o j", n=GR, o=1),
                            p4[:, :, 0:1, :], p4[:, :, 1:2, :])
                        # b += t on DVE: mixed f32+fp16 is 1x but tiny, and
                        # avoids a cross-engine hop (Pool adds cost ~4us/op)
                        nc.vector.tensor_add(b_g[:], b_g[:], t16[:])
                        # core-local softmax over j (all J present).  e_grp is
                        # bf16, whose f32-sized exponent range makes the
                        # max-subtraction unnecessary (|logits| << 88), so exp
                        # depends only on b_g — one DVE op and one chain hop
                        # fewer.  Z comes free from the exp's accum_out; 1/Z
                        # is folded into the PE stationary ln_n = d1 * invz_n
                        # (bf16: invz can underflow fp16), built on ACT where
                        # AP-scalar reads are cheap.
                        e_grp = spg.tile([128, GR * J], bf16, tag="e_grp",
                                         bufs=5)
                        zacc = spg.tile([128, GR], f32, tag="zacc", bufs=5)
                        for n in range(GR):
                            nc.scalar.activation(
                                e_grp[:, n * J:(n + 1) * J],
                                b_g[:, n * J:(n + 1) * J],
                                Act.Exp, bias=0.0,
                                accum_out=zacc[:, n:n + 1])
                        # software pipeline: emit the previous group's
                        # prod2+matmuls BEFORE this group's recip/ln so the
                        # DVE queue never stalls on ACT's zacc readback
                        if pending is not None:
                            stage_b(*pending)
                        pending = (u_t, e_grp, zacc, g0)
                    stage_b(*pending)

    nc.compile()
    return nc


def _prep_inputs(x, w_ij):
    """Host-side shard + layout. Returns per-core in_maps."""
    x_t = np.ascontiguousarray(x.transpose(1, 2, 0)).astype(np.float16)   # [I,D,B]
    # (c, j) free layout: W arranged [I, D, C, J]
    w_t = np.ascontiguousarray(w_ij.transpose(1, 3, 2, 0)).astype(np.float16)
    d1 = np.tile(np.eye(B, dtype=np.float16), (G, 1))                     # [128,B]
    in_maps = []
    for k in range(NCORES):
        xs = x_t[k * I_LOC:(k + 1) * I_LOC].reshape(NBLK, G, D, B)
        xd = np.zeros((NBLK, G * D, 128), np.float16)
        for g in range(G):
            xd[:, g * D:(g + 1) * D, g * B:(g + 1) * B] = xs[:, g]
        ws = w_t[k * I_LOC:(k + 1) * I_LOC].reshape(NBLK // 2, 2 * G * D, JC)
        # 1/J folded here so iteration 1's s needs no extra scale pass
        xsk = np.ascontiguousarray(
            x_t[k * I_LOC:(k + 1) * I_LOC].reshape(NBLK // 2, 2 * G * D, B)
            .astype(np.float32) / J).astype(np.float16)
        in_maps.append({"xd": xd.reshape(NBLK // 2, 2 * G * D, 128),
                        "xs": xsk, "wm": np.ascontiguousarray(ws),
                        "d1": d1})
    return in_maps


def kernel(x, w_ij, _trace=False):
    from concourse import bass_utils

    if "nc" not in _CACHE:
        _CACHE["nc"] = _build()
    nc = _CACHE["nc"]
    in_maps = _prep_inputs(np.asarray(x), np.asarray(w_ij))
    res = bass_utils.run_bass_kernel_spmd(
        nc, in_maps, core_ids=list(range(NCORES)), trace=_trace)
    _CACHE["last_result"] = res
    # kernel returns [B, (C, J)]; transpose back to [B, J, C] on host
    v = res.results[0]["v_out"].reshape(B, C, J).transpose(0, 2, 1)
    return np.ascontiguousarray(v).astype(np.float32)
